# revision 22
# baseline (speedup 1.0000x reference)
"""MoE layer (top-k routing) on 8 Trainium2 NeuronCores.

Expert-parallel per the sharding hint: the host computes router softmax +
top-k (0.1% of FLOPs) and realizes the "all-to-all dispatch by expert
assignment" while building the per-core SPMD input maps; each core runs
expert FFN work in bf16 (fp32 PSUM accumulation); the host applies the
combine weights and scatter-adds results back to [B,N,C].

Load balance: each expert's FFN is split along D_FF into four quarter-units
(exact: gelu is elementwise over F and GEMM2 contracts F, so the four
partial y's just add). The 32 quarter-units are assigned four per core, one
per slot class A-D: slot A holds the two largest experts' quarters, slot B
the next two, etc. Each slot is padded to the max count within its pair, so
per-core padded work is sum over slots of max(pair) — within ~1% of the
perfect-balance floor — instead of 2*max(all counts).
"""

import json
import os
import sys
import types

import numpy as np
import ml_dtypes

D_MODEL = 1024
D_FF = 4096
N_EXPERTS = 8
N_CORES = 8

P = 128
CB = D_MODEL // P      # 8 c-blocks of 128
FQ = D_FF // 4         # F quarter = 1024
FBQ = FQ // P          # 8 f-blocks per quarter
TN = 512               # token tile (matmul moving free dim / one PSUM bank)
SLOTS = ("A", "B", "C", "D")


def _shim_axon_hooks():
    """Register the NTFF profile hook bass_utils looks for under axon; the
    image's `antenv` stub lacks `axon_hooks`."""
    if "antenv.axon_hooks" in sys.modules:
        return
    try:
        import trn_agent_boot.trn_boot as _tb
        hook = _tb._ntff_profile_via_ctypes("/opt/axon/libaxon_pjrt.so")
    except Exception:
        hook = None
    mod = types.ModuleType("antenv.axon_hooks")
    mod.get_axon_ntff_profile_hook = lambda: hook
    mod.set_axon_ntff_profile_hook = lambda h: None
    sys.modules["antenv.axon_hooks"] = mod


_shim_axon_hooks()

import concourse.bass as bass            # noqa: E402
import concourse.tile as tile            # noqa: E402
from concourse import mybir              # noqa: E402
from concourse.bass import ds, ts        # noqa: E402
from concourse.bass_utils import run_bass_kernel_spmd  # noqa: E402


def _fix_multiwait_bir(nc):
    """Split instructions carrying >1 sync wait (the TileContext tail drain)
    into single-wait NoOps; this walrus build rejects multi-wait CTRL
    instructions."""
    raw = bass.Bass.to_json_bytes(nc)
    d = json.loads(raw)
    for f in d["functions"]:
        for b in f["blocks"]:
            out = []
            for i in b["instructions"]:
                si = i.get("sync_info") or {}
                waits = si.get("on_wait") or []
                if len(waits) > 1:
                    for k, w in enumerate(waits[:-1]):
                        out.append({
                            "name": f"{i['name']}_wsplit{k}",
                            "engine": i["engine"],
                            "ins": [], "outs": [],
                            "opcode": "NoOp",
                            "sync_info": {"on_update": [], "on_wait": [w]},
                        })
                    si["on_wait"] = [waits[-1]]
                out.append(i)
            b["instructions"] = out
    fixed = json.dumps(d).encode()
    nc.to_json_bytes = lambda: fixed


_NC_CACHE = {}


def _token_tiles(cap, small_first=False):
    # small_first: a 256-token leading tile halves the bytes the very first
    # matmul waits on; later tiles are prefetched early enough to cover it
    tiles, off = [], 0
    if small_first and cap > TN:
        tiles.append((0, 256))
        off = 256
    while off < cap:
        tw = min(TN, cap - off)
        tiles.append((off, tw))
        off += tw
    return tiles


def _build_moe_kernel(caps):
    """Four quarter-expert FFN units per core (slots A-D), SPMD x8."""
    key = tuple(caps)
    if key in _NC_CACHE:
        return _NC_CACHE[key]

    bf16 = mybir.dt.bfloat16
    f32 = mybir.dt.float32
    Act = mybir.ActivationFunctionType

    nc = bass.Bass("TRN2", target_bir_lowering=False, debug=False,
                   num_devices=N_CORES)

    units = []
    for slot, cap in zip(SLOTS, caps):
        u = {"cap": cap, "slot": slot}
        u["xT"] = nc.declare_dram_parameter(f"xT{slot}", [D_MODEL, cap], bf16, isOutput=False)
        u["w1t"] = nc.declare_dram_parameter(f"w1t{slot}", [D_MODEL, FQ], bf16, isOutput=False)
        u["w2t"] = nc.declare_dram_parameter(f"w2t{slot}", [FQ, D_MODEL], bf16, isOutput=False)
        u["b1"] = nc.declare_dram_parameter(f"b1{slot}", [FQ], f32, isOutput=False)
        u["b2"] = nc.declare_dram_parameter(f"b2{slot}", [D_MODEL], f32, isOutput=False)
        # partials return as bf16: halves the output DMA so total traffic
        # stays under the chip's P0 power-throttle trigger (observed: the
        # f32 version pushed PE from 2.4 to 2.0 GHz); host sums in f32
        u["yT"] = nc.declare_dram_parameter(f"yT{slot}", [D_MODEL, cap], bf16, isOutput=True)
        u["xr"] = u["xT"].ap().rearrange("(g p) t -> p g t", p=P)     # [128, 8, cap]
        u["w1r"] = u["w1t"].ap().rearrange("(g p) f -> p g f", p=P)   # [128, 8, 1024]
        u["w2r"] = u["w2t"].ap().rearrange("(g p) c -> p g c", p=P)   # [128, 8, 1024]
        u["b1r"] = u["b1"].ap().rearrange("(g p) -> p g", p=P)        # [128, 8]
        u["b2r"] = u["b2"].ap().rearrange("(g p) -> p g", p=P)        # [128, 8]
        u["yr"] = u["yT"].ap().rearrange("(g p) t -> p g t", p=P)
        u["tiles"] = _token_tiles(cap)
        units.append(u)

    with tile.TileContext(nc) as tc:
        with (
            tc.tile_pool(name="weights", bufs=1) as wpool,
            tc.tile_pool(name="xin", bufs=3) as xpool,
            tc.tile_pool(name="hbuf", bufs=1) as hpool,
            tc.tile_pool(name="yout", bufs=2) as ypool,
            tc.tile_pool(name="psum", bufs=4, space="PSUM") as psum,
        ):
            # ---- loads. SP-ring FIFO order is chosen so PE never waits:
            # tile-0 tokens + first w1 strip first (w1 strip on the ACT ring
            # so it overlaps x0's load), then unit A's remaining weights,
            # then unit A's tile-1 tokens BEFORE units B-D's weight bulk so
            # early tiles stay ahead of the PE.
            ua = units[0]
            a_tw0 = ua["tiles"][0][1]
            ua["x0"] = xpool.tile([P, CB, TN], bf16, tag="xt", name="x0A")
            nc.sync.dma_start(ua["x0"][:, :, :a_tw0], ua["xr"][:, :, ds(0, a_tw0)])
            ua["w1_sb"] = wpool.tile([P, CB, FQ], bf16, tag="w1A", name="w1A")
            nc.scalar.dma_start(ua["w1_sb"][:, :, 0:512], ua["w1r"][:, :, 0:512])

            # timing-gated PE pre-warm: tiny matmuls that READ the just-
            # landed x0A tile (RAW dep on its DMA) keep the PE busy through
            # the HAM activity window right before the first real matmul,
            # opening the 1.2->2.4GHz clock gate without being able to run
            # ahead of the data or delay the strip0-gated real stream
            warmw = wpool.tile([P, P], bf16, tag="warm", name="warm")
            nc.vector.memset(warmw[:], 0)
            for _ in range(30):
                pw = psum.tile([P, TN], f32, tag="ph")
                nc.tensor.matmul(pw[:, :16], lhsT=warmw[:, :],
                                 rhs=ua["x0"][:, 0, :16], start=True, stop=True)

            # only unit A's biases are needed early (first gelu / first
            # GEMM2 epilogue); each small DMA costs ~0.8us of SP-queue time,
            # so units B-D biases go after x1A
            ua["b1_sb"] = wpool.tile([P, FBQ], f32, tag="b1A", name="b1A")
            nc.sync.dma_start(ua["b1_sb"][:], ua["b1r"])
            ua["b2_sb"] = wpool.tile([P, CB], f32, tag="b2A", name="b2A")
            nc.sync.dma_start(ua["b2_sb"][:], ua["b2r"])

            nc.sync.dma_start(ua["w1_sb"][:, :, 512:FQ], ua["w1r"][:, :, 512:FQ])
            ua["w2_sb"] = wpool.tile([P, FBQ, D_MODEL], bf16, tag="w2A", name="w2A")
            nc.sync.dma_start(ua["w2_sb"][:, :, :], ua["w2r"][:, :, :])

            # unit A tile 1 tokens ahead of the remaining weight bulk
            if len(ua["tiles"]) > 1:
                o1, t1 = ua["tiles"][1]
                ua["x1"] = xpool.tile([P, CB, TN], bf16, tag="xt", name="x1A")
                nc.sync.dma_start(ua["x1"][:, :, :t1], ua["xr"][:, :, ds(o1, t1)])

            for u in units[1:]:
                u["b1_sb"] = wpool.tile([P, FBQ], f32, tag=f"b1{u['slot']}",
                                        name=f"b1{u['slot']}")
                nc.sync.dma_start(u["b1_sb"][:], u["b1r"])
                u["b2_sb"] = wpool.tile([P, CB], f32, tag=f"b2{u['slot']}",
                                        name=f"b2{u['slot']}")
                nc.sync.dma_start(u["b2_sb"][:], u["b2r"])

            for u in units[1:]:
                slot = u["slot"]
                u["w1_sb"] = wpool.tile([P, CB, FQ], bf16, tag=f"w1{slot}",
                                        name=f"w1{slot}")
                nc.sync.dma_start(u["w1_sb"][:, :, :], u["w1r"][:, :, :])
                u["w2_sb"] = wpool.tile([P, FBQ, D_MODEL], bf16, tag=f"w2{slot}",
                                        name=f"w2{slot}")
                nc.sync.dma_start(u["w2_sb"][:, :, :], u["w2r"][:, :, :])

            # ---- compute: unit A..D tiles in sequence ----
            for u in units:
                for ti, (off, tw) in enumerate(u["tiles"]):
                    if ti == 0 and "x0" in u:
                        xt = u["x0"]
                    elif ti == 1 and "x1" in u:
                        xt = u["x1"]
                    else:
                        xt = xpool.tile([P, CB, TN], bf16, tag="xt")
                        nc.sync.dma_start(xt[:, :, :tw], u["xr"][:, :, ds(off, tw)])

                    ht = hpool.tile([P, FBQ, TN], bf16, tag="ht")
                    for m in range(FBQ):
                        ph = psum.tile([P, TN], f32, tag="ph")
                        for k in range(CB):
                            nc.tensor.matmul(
                                ph[:, :tw],
                                lhsT=u["w1_sb"][:, k, ts(m, P)],
                                rhs=xt[:, k, :tw],
                                start=(k == 0), stop=(k == CB - 1),
                            )
                        nc.scalar.activation(ht[:, m, :tw], ph[:, :tw], Act.Gelu,
                                             bias=u["b1_sb"][:, m:m + 1])

                    last = (u is units[-1]) and (ti == len(u["tiles"]) - 1)
                    yt = ypool.tile([P, CB, TN], bf16, tag="yt")
                    for c in range(CB):
                        py = psum.tile([P, TN], f32, tag="py")
                        for k in range(FBQ):
                            nc.tensor.matmul(
                                py[:, :tw],
                                lhsT=u["w2_sb"][:, k, ts(c, P)],
                                rhs=ht[:, k, :tw],
                                start=(k == 0), stop=(k == FBQ - 1),
                            )
                        nc.scalar.add(yt[:, c, :tw], py[:, :tw], u["b2_sb"][:, c:c + 1])
                        if last:
                            # final tile: per-block stores overlap the tail
                            # GEMM2 blocks instead of one post-loop DMA
                            nc.sync.dma_start(u["yr"][:, c, ds(off, tw)],
                                              yt[:, c, :tw])
                    if not last:
                        nc.sync.dma_start(u["yr"][:, :, ds(off, tw)], yt[:, :, :tw])

    _fix_multiwait_bir(nc)
    _NC_CACHE[key] = nc
    return nc


def _route(xf, router_w, k):
    """Replicate the reference router numerics (f32 softmax, top-k, renorm)."""
    logits = xf @ router_w.T.astype(np.float32)          # [T, E]
    m = logits.max(axis=-1, keepdims=True)
    e = np.exp(logits - m, dtype=np.float32)
    probs = e / e.sum(axis=-1, keepdims=True)
    # descending, ties -> lower index first (matches jax.lax.top_k)
    idx = np.argsort(-probs, axis=-1, kind="stable")[:, :k]   # [T, k]
    w = np.take_along_axis(probs, idx, axis=-1)               # [T, k]
    w = w / (w.sum(axis=-1, keepdims=True) + 1e-9)
    return idx, w


def _align16(n):
    return max(P, -(-n // 16) * 16)


def kernel(x, router_w, expert_w1, expert_b1, expert_w2, expert_b2, top_k):
    x = np.asarray(x)
    router_w = np.asarray(router_w, dtype=np.float32)
    expert_w1 = np.asarray(expert_w1, dtype=np.float32)
    expert_b1 = np.asarray(expert_b1, dtype=np.float32)
    expert_w2 = np.asarray(expert_w2, dtype=np.float32)
    expert_b2 = np.asarray(expert_b2, dtype=np.float32)
    k = int(np.asarray(top_k))
    Bq, Nq, C = x.shape
    Tq = Bq * Nq
    E = expert_w1.shape[0]
    xf = np.ascontiguousarray(x.reshape(Tq, C), dtype=np.float32)

    idx, w = _route(xf, router_w, k)

    tok_idx, tok_w = [], []
    for e in range(E):
        mask = idx == e
        sel = np.nonzero(mask.any(axis=-1))[0]
        tok_idx.append(sel)
        tok_w.append((w * mask).sum(axis=-1)[sel].astype(np.float32))
    counts = np.array([len(s) for s in tok_idx])

    # slot s holds the quarters of the experts ranked 2s and 2s+1 by count;
    # cores 0-3 take quarters 0-3 of the first, cores 4-7 of the second
    order = np.argsort(-counts, kind="stable")
    caps = [_align16(int(counts[order[2 * s]])) for s in range(4)]

    nc = _build_moe_kernel(caps)

    # one xT per expert, shared by its four quarter-units
    xTs, slot_of = {}, {}
    for s in range(4):
        for j in (0, 1):
            e = int(order[2 * s + j])
            slot_of[e] = s
            xT = np.zeros((C, caps[s]), dtype=ml_dtypes.bfloat16)
            xT[:, :counts[e]] = xf[tok_idx[e]].T
            xTs[e] = xT

    in_maps = [dict() for _ in range(N_CORES)]
    placement = {}          # (expert, quarter) -> (core, slot name)
    for s, slot in enumerate(SLOTS):
        for core in range(N_CORES):
            e = int(order[2 * s + (0 if core < 4 else 1)])
            q = core % 4
            placement[(e, q)] = (core, slot)
            f0, f1 = q * FQ, (q + 1) * FQ
            b2 = expert_b2[e] if q == 0 else np.zeros(C, dtype=np.float32)
            in_maps[core].update({
                f"xT{slot}": xTs[e],
                f"w1t{slot}": np.ascontiguousarray(expert_w1[e, f0:f1].T).astype(ml_dtypes.bfloat16),
                f"w2t{slot}": np.ascontiguousarray(expert_w2[e, :, f0:f1].T).astype(ml_dtypes.bfloat16),
                f"b1{slot}": np.ascontiguousarray(expert_b1[e, f0:f1]),
                f"b2{slot}": np.ascontiguousarray(b2),
            })

    trace = os.environ.get("BASS_MOE_TRACE") == "1"
    res = run_bass_kernel_spmd(
        nc, in_maps, core_ids=list(range(N_CORES)),
        trace=trace,
        tmpdir=os.environ.get("BASS_MOE_TMPDIR") if trace else None,
    )
    if trace:
        kernel.last_exec_time_ns = res.exec_time_ns
        kernel.last_trace = (res.instructions_and_trace or (None, None))[1]

    out = np.zeros((Tq, C), dtype=np.float32)
    for e in range(E):
        cnt = counts[e]
        if not cnt:
            continue
        acc = np.zeros((cnt, C), dtype=np.float32)
        for q in range(4):
            core, slot = placement[(e, q)]
            acc += res.results[core][f"yT{slot}"][:, :cnt].T.astype(np.float32)
        out[tok_idx[e]] += acc * tok_w[e][:, None]
    return out.reshape(Bq, Nq, C).astype(x.dtype)


# revision 24
# speedup vs baseline: 1.0019x; 1.0019x over previous
"""MoE layer (top-k routing) on 8 Trainium2 NeuronCores.

Expert-parallel per the sharding hint: the host computes router softmax +
top-k (0.1% of FLOPs) and realizes the "all-to-all dispatch by expert
assignment" while building the per-core SPMD input maps; each core runs
expert FFN work in bf16 (fp32 PSUM accumulation); the host applies the
combine weights and scatter-adds results back to [B,N,C].

Load balance: each expert's FFN is split along D_FF into four quarter-units
(exact: gelu is elementwise over F and GEMM2 contracts F, so the four
partial y's just add). The 32 quarter-units are assigned four per core, one
per slot class A-D: slot A holds the two largest experts' quarters, slot B
the next two, etc. Each slot is padded to the max count within its pair, so
per-core padded work is sum over slots of max(pair) — within ~1% of the
perfect-balance floor — instead of 2*max(all counts).
"""

import json
import os
import sys
import types

import numpy as np
import ml_dtypes

D_MODEL = 1024
D_FF = 4096
N_EXPERTS = 8
N_CORES = 8

P = 128
CB = D_MODEL // P      # 8 c-blocks of 128
FQ = D_FF // 4         # F quarter = 1024
FBQ = FQ // P          # 8 f-blocks per quarter
TN = 512               # token tile (matmul moving free dim / one PSUM bank)
SLOTS = ("A", "B", "C", "D")


def _shim_axon_hooks():
    """Register the NTFF profile hook bass_utils looks for under axon; the
    image's `antenv` stub lacks `axon_hooks`."""
    if "antenv.axon_hooks" in sys.modules:
        return
    try:
        import trn_agent_boot.trn_boot as _tb
        hook = _tb._ntff_profile_via_ctypes("/opt/axon/libaxon_pjrt.so")
    except Exception:
        hook = None
    mod = types.ModuleType("antenv.axon_hooks")
    mod.get_axon_ntff_profile_hook = lambda: hook
    mod.set_axon_ntff_profile_hook = lambda h: None
    sys.modules["antenv.axon_hooks"] = mod


_shim_axon_hooks()

import concourse.bass as bass            # noqa: E402
import concourse.tile as tile            # noqa: E402
from concourse import mybir              # noqa: E402
from concourse.bass import ds, ts        # noqa: E402
from concourse.bass_utils import run_bass_kernel_spmd  # noqa: E402


def _fix_multiwait_bir(nc):
    """Split instructions carrying >1 sync wait (the TileContext tail drain)
    into single-wait NoOps; this walrus build rejects multi-wait CTRL
    instructions."""
    raw = bass.Bass.to_json_bytes(nc)
    d = json.loads(raw)
    for f in d["functions"]:
        for b in f["blocks"]:
            out = []
            for i in b["instructions"]:
                si = i.get("sync_info") or {}
                waits = si.get("on_wait") or []
                if len(waits) > 1:
                    for k, w in enumerate(waits[:-1]):
                        out.append({
                            "name": f"{i['name']}_wsplit{k}",
                            "engine": i["engine"],
                            "ins": [], "outs": [],
                            "opcode": "NoOp",
                            "sync_info": {"on_update": [], "on_wait": [w]},
                        })
                    si["on_wait"] = [waits[-1]]
                out.append(i)
            b["instructions"] = out
    fixed = json.dumps(d).encode()
    nc.to_json_bytes = lambda: fixed


_NC_CACHE = {}


def _token_tiles(cap, small_first=False):
    # small_first: a 256-token leading tile halves the bytes the very first
    # matmul waits on; later tiles are prefetched early enough to cover it
    tiles, off = [], 0
    if small_first and cap > TN:
        tiles.append((0, 256))
        off = 256
    while off < cap:
        tw = min(TN, cap - off)
        tiles.append((off, tw))
        off += tw
    return tiles


def _build_moe_kernel(caps):
    """Four quarter-expert FFN units per core (slots A-D), SPMD x8."""
    key = tuple(caps)
    if key in _NC_CACHE:
        return _NC_CACHE[key]

    bf16 = mybir.dt.bfloat16
    f32 = mybir.dt.float32
    Act = mybir.ActivationFunctionType

    nc = bass.Bass("TRN2", target_bir_lowering=False, debug=False,
                   num_devices=N_CORES)

    units = []
    for slot, cap in zip(SLOTS, caps):
        u = {"cap": cap, "slot": slot}
        u["xT"] = nc.declare_dram_parameter(f"xT{slot}", [D_MODEL, cap], bf16, isOutput=False)
        u["w1t"] = nc.declare_dram_parameter(f"w1t{slot}", [D_MODEL, FQ], bf16, isOutput=False)
        u["w2t"] = nc.declare_dram_parameter(f"w2t{slot}", [FQ, D_MODEL], bf16, isOutput=False)
        u["b1"] = nc.declare_dram_parameter(f"b1{slot}", [FQ], f32, isOutput=False)
        u["b2"] = nc.declare_dram_parameter(f"b2{slot}", [D_MODEL], f32, isOutput=False)
        # partials return as bf16: halves the output DMA so total traffic
        # stays under the chip's P0 power-throttle trigger (observed: the
        # f32 version pushed PE from 2.4 to 2.0 GHz); host sums in f32
        u["yT"] = nc.declare_dram_parameter(f"yT{slot}", [D_MODEL, cap], bf16, isOutput=True)
        u["xr"] = u["xT"].ap().rearrange("(g p) t -> p g t", p=P)     # [128, 8, cap]
        u["w1r"] = u["w1t"].ap().rearrange("(g p) f -> p g f", p=P)   # [128, 8, 1024]
        u["w2r"] = u["w2t"].ap().rearrange("(g p) c -> p g c", p=P)   # [128, 8, 1024]
        u["b1r"] = u["b1"].ap().rearrange("(g p) -> p g", p=P)        # [128, 8]
        u["b2r"] = u["b2"].ap().rearrange("(g p) -> p g", p=P)        # [128, 8]
        u["yr"] = u["yT"].ap().rearrange("(g p) t -> p g t", p=P)
        u["tiles"] = _token_tiles(cap)
        units.append(u)

    with tile.TileContext(nc) as tc:
        with (
            tc.tile_pool(name="weights", bufs=1) as wpool,
            tc.tile_pool(name="xin", bufs=3) as xpool,
            tc.tile_pool(name="hbuf", bufs=1) as hpool,
            tc.tile_pool(name="yout", bufs=2) as ypool,
            tc.tile_pool(name="psum", bufs=4, space="PSUM") as psum,
        ):
            # ---- loads. SP-ring FIFO order is chosen so PE never waits:
            # tile-0 tokens + first w1 strip first (w1 strip on the ACT ring
            # so it overlaps x0's load), then unit A's remaining weights,
            # then unit A's tile-1 tokens BEFORE units B-D's weight bulk so
            # early tiles stay ahead of the PE.
            ua = units[0]
            a_tw0 = ua["tiles"][0][1]
            ua["x0"] = xpool.tile([P, CB, TN], bf16, tag="xt", name="x0A")
            nc.sync.dma_start(ua["x0"][:, :, :a_tw0], ua["xr"][:, :, ds(0, a_tw0)])
            ua["w1_sb"] = wpool.tile([P, CB, FQ], bf16, tag="w1A", name="w1A")
            nc.scalar.dma_start(ua["w1_sb"][:, :, 0:512], ua["w1r"][:, :, 0:512])

            # only unit A's biases are needed early (first gelu / first
            # GEMM2 epilogue); each small DMA costs ~0.8us of SP-queue time,
            # so units B-D biases go after x1A
            ua["b1_sb"] = wpool.tile([P, FBQ], f32, tag="b1A", name="b1A")
            nc.sync.dma_start(ua["b1_sb"][:], ua["b1r"])
            ua["b2_sb"] = wpool.tile([P, CB], f32, tag="b2A", name="b2A")
            nc.sync.dma_start(ua["b2_sb"][:], ua["b2r"])

            nc.sync.dma_start(ua["w1_sb"][:, :, 512:FQ], ua["w1r"][:, :, 512:FQ])
            ua["w2_sb"] = wpool.tile([P, FBQ, D_MODEL], bf16, tag="w2A", name="w2A")
            nc.sync.dma_start(ua["w2_sb"][:, :, :], ua["w2r"][:, :, :])

            # unit A tile 1 tokens ahead of the remaining weight bulk
            if len(ua["tiles"]) > 1:
                o1, t1 = ua["tiles"][1]
                ua["x1"] = xpool.tile([P, CB, TN], bf16, tag="xt", name="x1A")
                nc.sync.dma_start(ua["x1"][:, :, :t1], ua["xr"][:, :, ds(o1, t1)])

            for u in units[1:]:
                u["b1_sb"] = wpool.tile([P, FBQ], f32, tag=f"b1{u['slot']}",
                                        name=f"b1{u['slot']}")
                nc.sync.dma_start(u["b1_sb"][:], u["b1r"])
                u["b2_sb"] = wpool.tile([P, CB], f32, tag=f"b2{u['slot']}",
                                        name=f"b2{u['slot']}")
                nc.sync.dma_start(u["b2_sb"][:], u["b2r"])

            for u in units[1:]:
                slot = u["slot"]
                u["w1_sb"] = wpool.tile([P, CB, FQ], bf16, tag=f"w1{slot}",
                                        name=f"w1{slot}")
                nc.sync.dma_start(u["w1_sb"][:, :, :], u["w1r"][:, :, :])
                u["w2_sb"] = wpool.tile([P, FBQ, D_MODEL], bf16, tag=f"w2{slot}",
                                        name=f"w2{slot}")
                nc.sync.dma_start(u["w2_sb"][:, :, :], u["w2r"][:, :, :])

            # ---- compute: unit A..D tiles in sequence ----
            for u in units:
                for ti, (off, tw) in enumerate(u["tiles"]):
                    if ti == 0 and "x0" in u:
                        xt = u["x0"]
                    elif ti == 1 and "x1" in u:
                        xt = u["x1"]
                    else:
                        xt = xpool.tile([P, CB, TN], bf16, tag="xt")
                        nc.sync.dma_start(xt[:, :, :tw], u["xr"][:, :, ds(off, tw)])

                    ht = hpool.tile([P, FBQ, TN], bf16, tag="ht")
                    for m in range(FBQ):
                        ph = psum.tile([P, TN], f32, tag="ph")
                        for k in range(CB):
                            nc.tensor.matmul(
                                ph[:, :tw],
                                lhsT=u["w1_sb"][:, k, ts(m, P)],
                                rhs=xt[:, k, :tw],
                                start=(k == 0), stop=(k == CB - 1),
                            )
                        nc.scalar.activation(ht[:, m, :tw], ph[:, :tw], Act.Gelu,
                                             bias=u["b1_sb"][:, m:m + 1])

                    last = (u is units[-1]) and (ti == len(u["tiles"]) - 1)
                    yt = ypool.tile([P, CB, TN], bf16, tag="yt")
                    for c in range(CB):
                        py = psum.tile([P, TN], f32, tag="py")
                        for k in range(FBQ):
                            nc.tensor.matmul(
                                py[:, :tw],
                                lhsT=u["w2_sb"][:, k, ts(c, P)],
                                rhs=ht[:, k, :tw],
                                start=(k == 0), stop=(k == FBQ - 1),
                            )
                        nc.scalar.add(yt[:, c, :tw], py[:, :tw], u["b2_sb"][:, c:c + 1])
                        if last:
                            # final tile: per-block stores overlap the tail
                            # GEMM2 blocks instead of one post-loop DMA
                            nc.sync.dma_start(u["yr"][:, c, ds(off, tw)],
                                              yt[:, c, :tw])
                    if not last:
                        nc.sync.dma_start(u["yr"][:, :, ds(off, tw)], yt[:, :, :tw])

    _fix_multiwait_bir(nc)
    _NC_CACHE[key] = nc
    return nc


def _route(xf, router_w, k):
    """Replicate the reference router numerics (f32 softmax, top-k, renorm)."""
    logits = xf @ router_w.T.astype(np.float32)          # [T, E]
    m = logits.max(axis=-1, keepdims=True)
    e = np.exp(logits - m, dtype=np.float32)
    probs = e / e.sum(axis=-1, keepdims=True)
    # descending, ties -> lower index first (matches jax.lax.top_k)
    idx = np.argsort(-probs, axis=-1, kind="stable")[:, :k]   # [T, k]
    w = np.take_along_axis(probs, idx, axis=-1)               # [T, k]
    w = w / (w.sum(axis=-1, keepdims=True) + 1e-9)
    return idx, w


def _align16(n):
    # 8-token cap granularity: token dim is a free matmul dim; 8 tokens =
    # 16B bf16 rows keep DMA alignment while minimizing padded PE work
    return max(P, -(-n // 8) * 8)


def kernel(x, router_w, expert_w1, expert_b1, expert_w2, expert_b2, top_k):
    x = np.asarray(x)
    router_w = np.asarray(router_w, dtype=np.float32)
    expert_w1 = np.asarray(expert_w1, dtype=np.float32)
    expert_b1 = np.asarray(expert_b1, dtype=np.float32)
    expert_w2 = np.asarray(expert_w2, dtype=np.float32)
    expert_b2 = np.asarray(expert_b2, dtype=np.float32)
    k = int(np.asarray(top_k))
    Bq, Nq, C = x.shape
    Tq = Bq * Nq
    E = expert_w1.shape[0]
    xf = np.ascontiguousarray(x.reshape(Tq, C), dtype=np.float32)

    idx, w = _route(xf, router_w, k)

    tok_idx, tok_w = [], []
    for e in range(E):
        mask = idx == e
        sel = np.nonzero(mask.any(axis=-1))[0]
        tok_idx.append(sel)
        tok_w.append((w * mask).sum(axis=-1)[sel].astype(np.float32))
    counts = np.array([len(s) for s in tok_idx])

    # slot s holds the quarters of the experts ranked 2s and 2s+1 by count;
    # cores 0-3 take quarters 0-3 of the first, cores 4-7 of the second
    order = np.argsort(-counts, kind="stable")
    caps = [_align16(int(counts[order[2 * s]])) for s in range(4)]

    nc = _build_moe_kernel(caps)

    # one xT per expert, shared by its four quarter-units
    xTs, slot_of = {}, {}
    for s in range(4):
        for j in (0, 1):
            e = int(order[2 * s + j])
            slot_of[e] = s
            xT = np.zeros((C, caps[s]), dtype=ml_dtypes.bfloat16)
            xT[:, :counts[e]] = xf[tok_idx[e]].T
            xTs[e] = xT

    in_maps = [dict() for _ in range(N_CORES)]
    placement = {}          # (expert, quarter) -> (core, slot name)
    for s, slot in enumerate(SLOTS):
        for core in range(N_CORES):
            e = int(order[2 * s + (0 if core < 4 else 1)])
            q = core % 4
            placement[(e, q)] = (core, slot)
            f0, f1 = q * FQ, (q + 1) * FQ
            b2 = expert_b2[e] if q == 0 else np.zeros(C, dtype=np.float32)
            in_maps[core].update({
                f"xT{slot}": xTs[e],
                f"w1t{slot}": np.ascontiguousarray(expert_w1[e, f0:f1].T).astype(ml_dtypes.bfloat16),
                f"w2t{slot}": np.ascontiguousarray(expert_w2[e, :, f0:f1].T).astype(ml_dtypes.bfloat16),
                f"b1{slot}": np.ascontiguousarray(expert_b1[e, f0:f1]),
                f"b2{slot}": np.ascontiguousarray(b2),
            })

    trace = os.environ.get("BASS_MOE_TRACE") == "1"
    res = run_bass_kernel_spmd(
        nc, in_maps, core_ids=list(range(N_CORES)),
        trace=trace,
        tmpdir=os.environ.get("BASS_MOE_TMPDIR") if trace else None,
    )
    if trace:
        kernel.last_exec_time_ns = res.exec_time_ns
        kernel.last_trace = (res.instructions_and_trace or (None, None))[1]

    out = np.zeros((Tq, C), dtype=np.float32)
    for e in range(E):
        cnt = counts[e]
        if not cnt:
            continue
        acc = np.zeros((cnt, C), dtype=np.float32)
        for q in range(4):
            core, slot = placement[(e, q)]
            acc += res.results[core][f"yT{slot}"][:, :cnt].T.astype(np.float32)
        out[tok_idx[e]] += acc * tok_w[e][:, None]
    return out.reshape(Bq, Nq, C).astype(x.dtype)
